# revision 57
# baseline (speedup 1.0000x reference)
"""DGCNN on Trainium2 — self-contained kernel.

Data-parallel over graphs: 1024 graphs x 128 nodes, sharded 128 graphs/core
across 8 NeuronCores; each core computes its graphs fully locally, weights
replicated. Host packing is layout-only (shard slicing, dtype casts,
transposes of weight matrices, edge-index localization).

Per-graph adjacency counts come from one-hot edge matrices (one wide DVE
compare per graph, a few tiles offloaded to GPSIMD) contracted on the PE.
The GCN chain runs in f32 (the sort keys need f32-exact ordering), node
major and transpose-free: the symmetric normalization D^-1/2 C D^-1/2 is
applied as diag(u) folded into the propagate operand (ctu) plus a
per-partition activation scale. Feature persists for the pooling phase are
bf16. Emission is stage-sliced across blocks of DB graphs so every engine
sees independent work back-to-back.
"""


import numpy as np
import ml_dtypes
from contextlib import ExitStack

import concourse.bass as bass
import concourse.tile as tile
from concourse import bacc, mybir, masks
from concourse.bass_utils import run_bass_kernel_spmd

BF = mybir.dt.bfloat16
F32 = mybir.dt.float32
U16 = mybir.dt.uint16
I32 = mybir.dt.int32
AL = mybir.AluOpType
ACTF = mybir.ActivationFunctionType

NPG = 128          # nodes per graph
EPG = 2048         # random edges per graph (16 tiles of 128)
NT = EPG // 128    # 16 edge tiles
NS = 2 * NT        # one-hot slots per graph (src tiles then dst tiles)
HID = 128
KPOOL = 64
DB = 8             # graphs per pipeline block
PB = 8             # graphs per pooling block
POH = 5            # one-hot slots built on GPSIMD (rest on DVE)


def build_program(G, n_cores=8, reps=1):
    """Build the SPMD program for G graphs per core."""
    nc = bacc.Bacc("TRN2", target_bir_lowering=False, debug=False,
                   num_devices=n_cores)

    d = {}
    def din(name, shape, dt):
        d[name] = nc.dram_tensor(name, list(shape), dt, kind="ExternalInput").ap()
        return d[name]

    din("edges", (128, G * NS), BF)      # [p, g*NS+s]: s<16 src tile s, else dst
    din("edgf", (128, G * POH), F32)     # f32 copy of slots 0..POH-1 (gpsimd)
    din("xt", (3, G * NPG), F32)         # x transposed per core (hidden-major)
    din("w0", (3, HID), F32)
    din("w1", (HID, HID), F32)
    din("w2", (HID, HID), F32)
    din("w3b", (128, HID), F32)          # w3^T replicated on all partitions
    din("b0", (HID, 1), F32)
    din("b1", (HID, 1), F32)
    din("b2", (HID, 1), F32)
    din("b3c", (HID, 1), F32)
    din("w1c", (128, 3 * 16), BF)        # conv1 chunks k=0..2: [:, 16k:16k+16]
    din("w1c3", (1, 16), BF)             # conv1 chunk row 384
    din("c1b", (16, 1), F32)
    din("w2j", (16, 5 * 32), BF)         # conv2 slices j: [:, 32j:32j+32]
    din("c2b", (32, 1), F32)
    din("m1p", (32, 28 * 128), BF)       # mlp1: p-slices [:, 128p:128p+128]
    din("mb1", (HID, 1), F32)
    din("w2m", (HID, 5), BF)
    din("mb2", (5, 1), F32)
    out_dram = nc.dram_tensor("out", [5, G], F32, kind="ExternalOutput").ap()

    with tile.TileContext(nc) as tc, ExitStack() as ctx:
        build_body(ctx, tc, d, out_dram, G, reps)

    nc.compile()
    return nc


def build_body(ctx, tc, d, out_dram, G, reps, dbg=None):
    nc = tc.nc

    consts = ctx.enter_context(tc.tile_pool(name="consts", bufs=1))
    persist = ctx.enter_context(tc.tile_pool(name="persist", bufs=1))

    # ---- load inputs into SBUF ----
    EDG = consts.tile([128, G * NS], BF)
    nc.sync.dma_start(EDG[:], d["edges"][:])
    EDGF = consts.tile([128, G * POH], F32)
    nc.sync.dma_start(EDGF[:], d["edgf"][:])

    def load(name, shape, dt):
        t = consts.tile(list(shape), dt, tag=name)
        nc.sync.dma_start(t[:], d[name][:])
        return t
    W0 = load("w0", (3, HID), F32)
    W1 = load("w1", (HID, HID), F32)
    W2 = load("w2", (HID, HID), F32)
    W3B = load("w3b", (128, HID), F32)
    B0 = load("b0", (HID, 1), F32)
    B1 = load("b1", (HID, 1), F32)
    B2 = load("b2", (HID, 1), F32)
    B3C = load("b3c", (HID, 1), F32)
    W1C = load("w1c", (128, 48), BF)
    W1C3 = load("w1c3", (1, 16), BF)
    C1B = load("c1b", (16, 1), F32)
    W2J = load("w2j", (16, 160), BF)
    C2B = load("c2b", (32, 1), F32)
    M1P = load("m1p", (32, 28 * 128), BF)
    MB1 = load("mb1", (HID, 1), F32)
    W2M = load("w2m", (HID, 5), BF)
    MB2 = load("mb2", (5, 1), F32)

    # ---- constants built on device ----
    IDENT = consts.tile([128, 128], BF)
    masks.make_identity(nc, IDENT[:])
    IDENTF = consts.tile([128, 128], F32)
    masks.make_identity(nc, IDENTF[:])
    # IOTAROW[p, v] = v; IOTANS[p, v*NS+s] = v (merged one-hot compare)
    IOTAMI = consts.tile([128, 128], I32)
    nc.gpsimd.iota(IOTAMI[:], pattern=[[1, 128]], base=0, channel_multiplier=0)
    IOTAROW = consts.tile([128, 128], BF)
    nc.vector.tensor_copy(IOTAROW[:], IOTAMI[:])
    IOTANS = consts.tile([128, 128 * NS], BF)
    nc.vector.tensor_copy(
        IOTANS[:].rearrange("p (v s) -> p v s", s=NS),
        IOTAROW[:][:, :, None].broadcast_to([128, 128, NS]))
    IOTA32 = consts.tile([128, 1], I32)
    nc.gpsimd.iota(IOTA32[:], pattern=[[0, 1]], base=0, channel_multiplier=1)
    IOTACOL = consts.tile([128, 1], F32)
    nc.vector.tensor_copy(IOTACOL[:], IOTA32[:])
    ONESCOLF = consts.tile([128, 1], F32)
    nc.vector.memset(ONESCOLF[:], 1.0)
    ZJUNK = consts.tile([128, 128], F32)

    # ---- persistent per-graph state ----
    H1 = persist.tile([128, G * 128], BF)      # node-major tanh outputs
    H2 = persist.tile([128, G * 128], BF)
    H3 = persist.tile([128, G * 128], BF)
    H4C = persist.tile([128, max(G, 2)], BF)
    H4F = persist.tile([128, max(G, 2)], F32)
    USB = persist.tile([128, max(G, 2)], F32)  # u = rsqrt(deg) per graph col
    KEYS = persist.tile([128, NPG], F32)       # [graph, node] layout
    nc.vector.memset(KEYS[:], 0.0)
    Y2ALL = persist.tile([32, G * 28], BF)
    OUTSB = persist.tile([5, max(G, 2)], F32)

    # ---- rotating pools ----
    ohp = ctx.enter_context(tc.tile_pool(name="oh", bufs=2))        # [128,4096] bf16
    xgp = ctx.enter_context(tc.tile_pool(name="xg", bufs=2))        # [3,DB*128] f32
    ctfp = ctx.enter_context(tc.tile_pool(name="ctf", bufs=2 * DB + 2))
    ctup = ctx.enter_context(tc.tile_pool(name="ctu", bufs=DB + 2))
    sbp = ctx.enter_context(tc.tile_pool(name="sbwork", bufs=2 * DB - 2))
    p3p = ctx.enter_context(tc.tile_pool(name="p3work", bufs=3))
    smallp = ctx.enter_context(tc.tile_pool(name="small", bufs=4))
    psR = ctx.enter_context(tc.tile_pool(name="psR", bufs=8, space="PSUM"))

    def pr(shape, name):
        return psR.tile(list(shape), F32, tag="r", name=name)

    def body():
        # ============ phase 1: adjacency counts + u + GCN ============
        ctsfs = {}
        xgs = {}

        def s_counts(g):
            # one-hot build: one wide DVE op (slots POH..NS) + POH gpsimd
            # per-tile ops; slots 0..15 compare src ids, 16..31 dst ids.
            oh = ohp.tile([128, 128 * NS], BF, tag="oh")
            ohv = oh[:].rearrange("p (v s) -> p v s", s=NS)
            ev = EDG[:, g * NS + POH:(g + 1) * NS][:, None, :]
            nc.vector.tensor_tensor(
                ohv[:, :, POH:], IOTANS[:].rearrange(
                    "p (v s) -> p v s", s=NS)[:, :, POH:],
                ev.broadcast_to([128, 128, NS - POH]), AL.is_equal)
            for s in range(POH):
                nc.gpsimd.tensor_scalar(
                    ohv[:, :, s], IOTAROW[:],
                    EDGF[:, g * POH + s:g * POH + s + 1], None, AL.is_equal)

            cps = pr([128, 128], "cnt")
            for t in range(NT):
                nc.tensor.matmul(cps[:], ohv[:, :, t], ohv[:, :, NT + t],
                                 start=(t == 0), stop=False)
            nc.tensor.matmul(cps[:], IDENT[:], IDENT[:], start=False, stop=True)
            ctsf = ctfp.tile([128, 128], F32, tag="ctsf")
            nc.scalar.copy(ctsf[:], cps[:])
            ctsfs[g] = ctsf

        degpair = [None]

        def counts_chunks(b):
            """Emission chunks for block b's counts (interleavable)."""
            g0 = b * DB
            chunks = []

            def dma_chunk(b=b, g0=g0):
                xg = xgp.tile([3, DB * NPG], F32, tag="xg", name="xg")
                nc.sync.dma_start(xg[:], d["xt"][:, g0 * NPG:(g0 + DB) * NPG])
                xgs[b] = xg
            chunks.append(dma_chunk)

            def graph_chunk(b=b, j=0, g=0):
                if j == 0 and b % 2 == 0:
                    degpair[0] = pr([128, 2 * DB], "deg")
                s_counts(g)
                nc.tensor.matmul(degpair[0][:, (b % 2) * DB + j:
                                             (b % 2) * DB + j + 1],
                                 ctsfs[g][:], ONESCOLF[:],
                                 start=True, stop=True)
            for j in range(DB):
                chunks.append(lambda b=b, j=j, g=g0 + j: graph_chunk(b, j, g))

            if b % 2 == 1 or b == nblk - 1:
                def sqrt_chunk(b=b):
                    n = (b % 2) * DB + DB
                    gb0 = (b - b % 2) * DB
                    rec = smallp.tile([128, 2 * DB], F32, tag="rec",
                                      name="rec", bufs=2)
                    nc.vector.reciprocal(rec[:, :n], degpair[0][:, :n])
                    nc.scalar.activation(USB[:, gb0:gb0 + n], rec[:, :n],
                                         ACTF.Sqrt)
                chunks.append(sqrt_chunk)
            return chunks

        def layer_stages(b):
            """Stage-sliced GCN emission closures for block b."""
            g0 = b * DB
            gs = list(range(g0, g0 + DB))
            u_ = {g: USB[:, g:g + 1] for g in gs}
            ctu, t1, y1, p1, hf, q, qsb, t_ = {}, {}, {}, {}, {}, {}, {}, {}
            zcols = {}
            st = []

            def stage_ctu():
                ctsf = {g: ctsfs.pop(g) for g in gs}
                for g in gs:
                    ctu[g] = ctup.tile([128, 128], F32, tag="ctu", name="ctu")
                    nc.scalar.activation(ctu[g][:], ctsf[g][:], ACTF.Identity,
                                         scale=u_[g])
            st.append(stage_ctu)

            def stage_t1():
                xg = xgs.pop(b)
                for j, g in enumerate(gs):
                    t1[g] = pr([128, 128], "t1")
                    nc.tensor.matmul(t1[g][:], xg[:, j * NPG:(j + 1) * NPG],
                                     W0[:], start=True, stop=True)
            st.append(stage_t1)

            def stage_y1():
                for g in gs:
                    y1[g] = sbp.tile([128, 128], F32, tag="y1", name="y1",
                                     bufs=DB + 2)
                    nc.scalar.copy(y1[g][:], t1[g][:])
            st.append(stage_y1)

            def stage_p1():
                for g in gs:
                    p1[g] = pr([128, 128], "p1")
                    nc.tensor.matmul(p1[g][:], ctu[g][:], y1[g][:],
                                     start=True, stop=True)
            st.append(stage_p1)

            def stage_tanh1():
                for g in gs:
                    hf[g] = sbp.tile([128, 128], F32, tag="hf", name="hf")
                    nc.scalar.activation(hf[g][:], p1[g][:], ACTF.Tanh,
                                         bias=B0[:], scale=u_[g])
            st.append(stage_tanh1)

            def stage_h1copy():
                for g in gs:
                    nc.gpsimd.tensor_copy(H1[:, g * 128:(g + 1) * 128],
                                          hf[g][:])
            st.append(stage_h1copy)

            for l, (W_, B_, HPout) in enumerate(((W1, B1, H2), (W2, B2, H3))):
                def stage_q(l=l):
                    for g in gs:
                        q[g] = pr([128, 128], "q")
                        nc.tensor.matmul(q[g][:], hf[g][:], ctu[g][:],
                                         start=True, stop=True)
                st.append(stage_q)

                def stage_qsb(l=l):
                    for g in gs:
                        qsb[g] = sbp.tile([128, 128], F32, tag="qsb",
                                          name="qsb", bufs=DB + 2)
                        if l == 0:
                            nc.scalar.copy(qsb[g][:], q[g][:])
                        else:
                            nc.vector.tensor_copy(qsb[g][:], q[g][:])
                st.append(stage_qsb)

                def stage_t(l=l, W_=W_):
                    for g in gs:
                        t_[g] = pr([128, 128], "t")
                        nc.tensor.matmul(t_[g][:], qsb[g][:], W_[:],
                                         start=True, stop=True)
                st.append(stage_t)

                def stage_tanh(l=l, B_=B_):
                    for g in gs:
                        hf[g] = sbp.tile([128, 128], F32, tag="hf", name="hf")
                        nc.scalar.activation(hf[g][:], t_[g][:], ACTF.Tanh,
                                             bias=B_[:], scale=u_[g])
                st.append(stage_tanh)

                def stage_hcopy(l=l, HPout=HPout):
                    for g in gs:
                        nc.gpsimd.tensor_copy(
                            HPout[:, g * 128:(g + 1) * 128], hf[g][:])
                st.append(stage_hcopy)

            def stage_z():
                for g in gs:
                    zcols[g] = smallp.tile([128, 1], F32, tag="zcol",
                                           name="zcol", bufs=DB + 2)
                    nc.vector.scalar_tensor_tensor(ZJUNK[:], hf[g][:], 1.0,
                                                   W3B[:], AL.mult, AL.mult,
                                                   accum_out=zcols[g][:])
            st.append(stage_z)

            def stage_p4():
                p4b = pr([128, DB], "p4")
                for j, g in enumerate(gs):
                    nc.tensor.matmul(p4b[:, j:j + 1], ctu[g][:], zcols[g][:],
                                     start=True, stop=True)
                p4s = smallp.tile([128, DB], F32, tag="p4s", name="p4s", bufs=2)
                nc.vector.tensor_tensor(p4s[:], p4b[:], USB[:, g0:g0 + DB],
                                        AL.mult)
                nc.scalar.activation(H4F[:, g0:g0 + DB], p4s[:], ACTF.Tanh,
                                     bias=B3C[:])
            st.append(stage_p4)
            return st

        assert G % DB == 0
        nblk = G // DB
        assert nblk % 2 == 0 or nblk == 1
        for c in counts_chunks(0):
            c()
        if nblk > 1:
            for c in counts_chunks(1):
                c()
        for b in range(nblk):
            chunks = counts_chunks(b + 2) if b + 2 < nblk else []
            stages = layer_stages(b)
            ci = 0
            for i, stg in enumerate(stages):
                stg()
                while ci < len(chunks) and                         ci * len(stages) < (i + 1) * len(chunks):
                    chunks[ci]()
                    ci += 1
            for c in chunks[ci:]:
                c()

        # ============ phase 2: top-64 per graph ============
        nc.vector.tensor_copy(H4C[:], H4F[:])
        kt_ps = pr([128, 128], "kt")
        nc.tensor.transpose(kt_ps[:G, :], H4F[:, :G], IDENTF[:])
        nc.scalar.copy(KEYS[:G, :], kt_ps[:G, :])
        IDXU = persist.tile([128, 64], U16, tag="idxu")
        kcur = KEYS
        kalt = persist.tile([128, NPG], F32, tag="keys2")
        for r in range(8):
            mx = smallp.tile([128, 8], F32, tag="mx")
            nc.vector.max(mx[:], kcur[:])
            nc.vector.max_index(IDXU[:, 8 * r:8 * r + 8], mx[:], kcur[:])
            if r < 7:
                nc.vector.match_replace(kalt[:], mx[:], kcur[:], -1e30)
                kcur, kalt = kalt, kcur
        IDXFF = persist.tile([128, 64], F32, tag="idxff")
        nc.vector.tensor_copy(IDXFF[:], IDXU[:])
        idxt_ps = pr([64, 128], "idxt")
        nc.tensor.transpose(idxt_ps[:], IDXFF[:], IDENTF[:])
        IDXT = persist.tile([64, 128], F32, tag="idxt")
        nc.scalar.copy(IDXT[:], idxt_ps[:])

        # ============ phase 3: pool + convs, batched over PB graphs ============
        B = min(PB, G)
        assert G % B == 0

        def pool_stages(gb):
            pts = []
            c1h = {}
            y1h = {}
            st = []

            def stage_pt():
                ptall_ps = psR.tile([128, 64 * B], BF, tag="r", name="ptall")
                for gi in range(B):
                    g = gb + gi
                    ptt = p3p.tile([64, 128], BF, tag=f"ib{gi}", bufs=2,
                                   name="ptt")
                    nc.gpsimd.tensor_scalar(ptt[:], IOTAROW[0:64, :],
                                            IDXT[:, g:g + 1], None,
                                            AL.is_equal)
                    nc.tensor.transpose(ptall_ps[:, 64 * gi:64 * gi + 64],
                                        ptt[:], IDENT[0:64, 0:64])
                ptall = p3p.tile([128, 64 * B], BF, tag="ptall", bufs=2,
                                 name="ptall")
                nc.vector.tensor_copy(ptall[:], ptall_ps[:])
                for gi in range(B):
                    pts.append(ptall[:, 64 * gi:64 * gi + 64])
            st.append(stage_pt)

            def stage_convs():
                c1ps = pr([16, 64 * B], "c1")
                c1h[0] = c1ps
                chunk4 = pr([1, 64 * B], "c4")
                for gi in range(B):
                    nc.tensor.matmul(chunk4[0:1, 64 * gi:64 * gi + 64],
                                     H4C[:, gb + gi:gb + gi + 1], pts[gi][:],
                                     start=True, stop=True)
                c4sb = p3p.tile([1, 64 * B], BF, tag="c4sb", bufs=2,
                                name="c4sb")
                nc.scalar.copy(c4sb[:], chunk4[:])
                for l, HP in enumerate((H1, H2, H3)):
                    chunk = pr([128, 64 * B], "chunk")
                    for gi in range(B):
                        nc.tensor.matmul(
                            chunk[:, 64 * gi:64 * gi + 64],
                            HP[:, (gb + gi) * 128:(gb + gi + 1) * 128],
                            pts[gi], start=True, stop=True)
                    csb = p3p.tile([128, 64 * B], BF, tag="csb", bufs=2,
                                   name="csb")
                    (nc.scalar.copy if l == 2 else nc.vector.tensor_copy)(
                        csb[:], chunk[:])
                    nc.tensor.matmul(c1ps[:], W1C[:, 16 * l:16 * l + 16],
                                     csb[:], start=(l == 0), stop=False)
                nc.tensor.matmul(c1ps[:], W1C3[:], c4sb[:], start=False,
                                 stop=True)
            st.append(stage_convs)

            def stage_relu_pool():
                y1c = p3p.tile([16, 64 * B], BF, tag="y1c", bufs=2,
                               name="y1c")
                nc.scalar.activation(y1c[:], c1h[0][:], ACTF.Relu, bias=C1B[:])
                y1p = p3p.tile([16, 32 * B], BF, tag="y1p", name="y1p")
                y1v = y1c[:].rearrange("p (a b) -> p a b", b=2)
                nc.vector.tensor_tensor(y1p[:], y1v[:, :, 0], y1v[:, :, 1],
                                        AL.max)
                y1h[0] = y1p
            st.append(stage_relu_pool)

            def stage_conv2():
                c2ps = pr([32, 28 * B], "c2")
                y1pv = y1h[0][:].rearrange("p (g q) -> p g q", q=32)
                for j in range(5):
                    nc.tensor.matmul(c2ps[:], W2J[:, 32 * j:32 * j + 32],
                                     y1pv[:, :, j:j + 28], start=(j == 0),
                                     stop=(j == 4))
                nc.scalar.activation(Y2ALL[:, 28 * gb:28 * (gb + B)], c2ps[:],
                                     ACTF.Relu, bias=C2B[:])
            st.append(stage_conv2)
            return st

        blocks = list(range(0, G, B))
        slists = [pool_stages(gb) for gb in blocks]
        nstg = 4
        for t in range(len(blocks) + nstg - 1):
            for b in range(len(blocks)):
                k = t - b
                if 0 <= k < nstg:
                    slists[b][k]()

        # ============ phase 4: mlp, split in graph halves ============
        y2v = Y2ALL[:].rearrange("p (g q) -> p q g", q=28)
        GH = max(G // 2, 1)
        for h0 in range(0, G, GH):
            hm_ps = pr([128, max(GH, 2)], "hm")
            for p in range(28):
                nc.tensor.matmul(hm_ps[:, :GH], M1P[:, 128 * p:128 * p + 128],
                                 y2v[:, p, h0:h0 + GH], start=(p == 0),
                                 stop=(p == 27))
            HM = p3p.tile([128, GH], BF, tag="hm", bufs=2, name="HM")
            nc.scalar.activation(HM[:], hm_ps[:, :GH], ACTF.Relu, bias=MB1[:])
            ops = pr([5, max(GH, 2)], "ops")
            nc.tensor.matmul(ops[:, :GH], W2M[:], HM[:], start=True, stop=True)
            nc.scalar.activation(OUTSB[:, h0:h0 + GH], ops[:, :GH],
                                 ACTF.Identity, bias=MB2[:])

    if reps == 1:
        body()
    else:
        with tc.For_i(0, reps, 1):
            body()

    nc.sync.dma_start(out_dram[:], OUTSB[:, :G])
    if dbg is not None:
        for name, t in (("dH1", H1), ("dH2", H2), ("dH3", H3),
                        ("dH4", H4F), ("dU", USB), ("dY2", Y2ALL)):
            if name in dbg:
                nc.sync.dma_start(dbg[name][:], t[:])


# ================= host-side packing =================

def prep_core_inputs(inputs, core, G):
    """Pack the full problem inputs into per-core numpy arrays."""
    bf = ml_dtypes.bfloat16
    x = np.asarray(inputs["x"], np.float32)
    ei = np.asarray(inputs["edge_index"], np.int64)
    g0 = core * G
    n0 = g0 * NPG
    e0 = g0 * EPG

    def local_edges(row):
        loc = (row[e0:e0 + G * EPG].reshape(G, EPG)
               - (np.arange(g0, g0 + G, dtype=np.int64)[:, None] * NPG))
        assert loc.min() >= 0 and loc.max() < NPG, "edges not graph-local"
        return loc.reshape(G, NT, 128)          # [G, t, p]

    src = local_edges(ei[0])
    dst = local_edges(ei[1])
    # edges[p, g*NS+s]: s<NT -> src tile s, s>=NT -> dst tile s-NT
    edges = np.concatenate([src, dst], axis=1)   # [G, NS, 128]
    edges = np.ascontiguousarray(
        edges.transpose(2, 0, 1).reshape(128, G * NS)).astype(bf)

    w1c_full = np.asarray(inputs["conv1_w"], np.float32)[:, 0, :]  # [16, 385]
    w1c = np.concatenate([w1c_full[:, 128 * k:128 * k + 128].T for k in range(3)],
                         axis=1)  # [128, 48]
    w2j = np.concatenate([np.asarray(inputs["conv2_w"], np.float32)[:, :, j].T
                          for j in range(5)], axis=1)  # [16, 160]
    m1p = np.concatenate(
        [np.asarray(inputs["mlp_w1"], np.float32).reshape(32, 28, 128)[:, p, :]
         for p in range(28)], axis=1)  # [32, 28*128]

    edgf = np.ascontiguousarray(
        src[:, :POH].transpose(2, 0, 1).reshape(128, G * POH)).astype(np.float32)

    return {
        "edges": edges,
        "edgf": edgf,
        "xt": np.ascontiguousarray(x[n0:n0 + G * NPG].T),
        "w0": np.asarray(inputs["W0"], np.float32),
        "w1": np.asarray(inputs["W1"], np.float32),
        "w2": np.asarray(inputs["W2"], np.float32),
        "w3b": np.tile(np.asarray(inputs["W3"], np.float32).reshape(1, HID),
                       (128, 1)),
        "b0": np.asarray(inputs["b0"], np.float32).reshape(HID, 1),
        "b1": np.asarray(inputs["b1"], np.float32).reshape(HID, 1),
        "b2": np.asarray(inputs["b2"], np.float32).reshape(HID, 1),
        "b3c": np.full((HID, 1), float(np.asarray(inputs["b3"]).reshape(())),
                       np.float32),
        "w1c": w1c.astype(bf),
        "w1c3": w1c_full[:, 384:385].T.astype(bf),
        "c1b": np.asarray(inputs["conv1_b"], np.float32).reshape(16, 1),
        "w2j": w2j.astype(bf),
        "c2b": np.asarray(inputs["conv2_b"], np.float32).reshape(32, 1),
        "m1p": m1p.astype(bf),
        "mb1": np.asarray(inputs["mlp_b1"], np.float32).reshape(HID, 1),
        "w2m": np.asarray(inputs["mlp_w2"], np.float32).astype(bf),
        "mb2": np.asarray(inputs["mlp_b2"], np.float32).reshape(5, 1),
    }


def kernel(**inputs):
    """Full-inputs -> full-output entry point. 8 cores, 128 graphs each."""
    G, n_cores = 128, 8
    nc = build_program(G, n_cores=n_cores, reps=1)
    in_maps = [prep_core_inputs(inputs, c, G) for c in range(n_cores)]
    res = run_bass_kernel_spmd(nc, in_maps, core_ids=list(range(n_cores)))
    out = np.empty((n_cores * G, 5), np.float32)
    for c in range(n_cores):
        out[c * G:(c + 1) * G, :] = res.results[c]["out"].T
    return out
